# revision 10
# baseline (speedup 1.0000x reference)
"""Correlation-layer kernel for Trainium2 (8 NeuronCores, data-parallel over batch).

Problem (per batch b):
    corr[k, m] = sum_c x[b, c, u, v] * y[b, c, i, j],  k = v*h+u, m = i*w+j
    out = relu(corr) / sqrt(sum_k relu(corr)^2 + eps)   (normalize over k per m)

Shapes: x, y = (8, 128, 48, 64) fp32 -> out (8, 3072, 48, 64) fp32.
Sharding: 1 batch per core.

Design (v4): transposed layout — m on partitions, k on the free dim.
Per m-tile (128 rows of m, 3072 k columns):
  - 6 fp16 matmuls (lhsT = y columns, rhs = x) -> two 3-bank psum tiles.
  - relu evacuation psum -> fp16 r16 tile, split ACT/DVE.
  - sum-of-squares over k via one fused DVE tensor_tensor_reduce
    (accum_out, eps folded as the reduce initial value).
  - s = 1/sqrt(ss): ACT sqrt + DVE reciprocal (+1 Newton step).
  - out16 = r16 * s (per-partition scalar): DVE 4x tensor_scalar +
    GpSimd slice.
  - one contiguous 786 KB HWDGE DMA per m-tile into a [M, K] fp16
    DRAM tensor.
Host side: transpose [M, K] -> [K, M], upcast fp16 -> fp32, stack batches.
Normalization over k is a per-partition (free-dim) op in this layout, which
kills the ones-matmul reduction, transpose chains, and broadcast matmul of
the previous design, and fp16 output halves HBM write traffic.
"""

import sys

sys.path.insert(0, "/opt/trn_rl_repo")

import numpy as np

_BUILD_CACHE = {}

B, C, H, W = 8, 128, 48, 64
K = W * H      # 3072 output channels, k = v*h+u
M = H * W      # 3072 spatial positions, m = i*w+j
NT = M // 128  # 24 m-tiles
EPS = 1e-6

PSA = 1536     # cols in first psum tile (ACT evacs all of it)
PSB = 1536     # cols in second psum tile
EV_B_ACT = 1024  # leading cols of psB evac'd by ACT (bank-aligned), rest DVE
SC_GPS = 1536  # trailing cols of the scale pass on GpSimd (rest DVE)
NEWTON = True  # refine DVE reciprocal with one Newton step
USE_TTR = False  # fused square+reduce crashes on HW; use TT + tensor_reduce


def build():
    from concourse import bacc, bass, mybir, tile

    F32 = mybir.dt.float32
    F16 = mybir.dt.float16
    AF = mybir.ActivationFunctionType
    OP = mybir.AluOpType

    nc = bacc.Bacc("TRN2", debug=False, target_bir_lowering=False)

    a_d = nc.dram_tensor("a", [C, K], F16, kind="ExternalInput")
    b_d = nc.dram_tensor("b", [C, M], F16, kind="ExternalInput")
    out_d = nc.dram_tensor("out", [M, K], F16, kind="ExternalOutput")

    with tile.TileContext(nc) as tc:
        with (
            tc.tile_pool(name="pers", bufs=1) as pers,
            tc.tile_pool(name="rk", bufs=3) as rk,
            tc.tile_pool(name="sq", bufs=2) as sqp,
            tc.tile_pool(name="ok", bufs=3) as ok,
            tc.tile_pool(name="sm", bufs=4) as sm,
            tc.tile_pool(name="ps", bufs=1, space=bass.MemorySpace.PSUM) as ps,
        ):
            a_t = pers.tile([C, K], F16)
            b_t = pers.tile([C, M], F16)
            nc.sync.dma_start(b_t[:], b_d[:])
            nc.sync.dma_start(a_t[:], a_d[:])

            state = {}

            def emit_front(i):
                """matmuls + relu evac + fused square-reduce for m-tile i."""
                m0 = i * 128
                psA = ps.tile([128, PSA], F32, tag="psA")
                psB = ps.tile([128, PSB], F32, tag="psB")
                for j in range(PSA // 512):
                    nc.tensor.matmul(
                        psA[:, j * 512 : (j + 1) * 512],
                        b_t[:, m0 : m0 + 128],
                        a_t[:, j * 512 : (j + 1) * 512],
                        start=True, stop=True,
                    )
                for j in range(PSB // 512):
                    nc.tensor.matmul(
                        psB[:, j * 512 : (j + 1) * 512],
                        b_t[:, m0 : m0 + 128],
                        a_t[:, PSA + j * 512 : PSA + (j + 1) * 512],
                        start=True, stop=True,
                    )
                r16 = rk.tile([128, K], F16, tag="r16")
                nc.scalar.activation(r16[:, 0:PSA], psA[:], AF.Relu)
                if EV_B_ACT:
                    nc.scalar.activation(
                        r16[:, PSA : PSA + EV_B_ACT], psB[:, 0:EV_B_ACT], AF.Relu
                    )
                if EV_B_ACT < PSB:
                    nc.vector.tensor_scalar_max(
                        r16[:, PSA + EV_B_ACT : K], psB[:, EV_B_ACT : PSB], 0.0
                    )
                sq16 = sqp.tile([128, K], F16, tag="sq16")
                ss = sm.tile([128, 1], F32, tag="ss")
                if USE_TTR:
                    nc.vector.tensor_tensor_reduce(
                        out=sq16[:], in0=r16[:], in1=r16[:], scale=1.0, scalar=EPS,
                        op0=OP.mult, op1=OP.add, accum_out=ss[:],
                    )
                else:
                    nc.vector.tensor_tensor(sq16[:], r16[:], r16[:], OP.mult)
                    ss_r = sm.tile([128, 1], F32, tag="ssr")
                    nc.vector.tensor_reduce(
                        ss_r[:], sq16[:], mybir.AxisListType.X, OP.add
                    )
                    nc.vector.tensor_scalar_add(ss[:], ss_r[:], EPS)
                state[i] = (r16, ss)

            def emit_back(i):
                """rsqrt + scale + output DMA for m-tile i."""
                r16, ss = state.pop(i)
                s0 = sm.tile([128, 1], F32, tag="s0")
                nc.scalar.activation(s0[:], ss[:], AF.Sqrt)
                y0 = sm.tile([128, 1], F32, tag="y0")
                nc.vector.reciprocal(y0[:], s0[:])
                if NEWTON:
                    # y1 = y0 * (2 - s0*y0)
                    p = sm.tile([128, 1], F32, tag="p")
                    nc.vector.tensor_tensor(p[:], s0[:], y0[:], OP.mult)
                    q = sm.tile([128, 1], F32, tag="q")
                    nc.vector.tensor_scalar(
                        out=q[:], in0=p[:], scalar1=-1.0, scalar2=2.0,
                        op0=OP.mult, op1=OP.add,
                    )
                    s_fin = sm.tile([128, 1], F32, tag="sfin")
                    nc.vector.tensor_tensor(s_fin[:], y0[:], q[:], OP.mult)
                else:
                    s_fin = y0
                out16 = ok.tile([128, K], F16, tag="out16")
                nc.vector.tensor_scalar(
                    out=out16[:, 0 : K - SC_GPS], in0=r16[:, 0 : K - SC_GPS],
                    scalar1=s_fin[:], scalar2=None, op0=OP.mult,
                )
                if SC_GPS:
                    nc.gpsimd.tensor_scalar(
                        out=out16[:, K - SC_GPS : K], in0=r16[:, K - SC_GPS : K],
                        scalar1=s_fin[:], scalar2=None, op0=OP.mult,
                    )
                nc.sync.dma_start(out_d[i * 128 : (i + 1) * 128, :], out16[:])

            emit_front(0)
            for i in range(1, NT + 1):
                if i < NT:
                    emit_front(i)
                emit_back(i - 1)

    nc.compile()
    return nc


def get_built():
    if "nc" not in _BUILD_CACHE:
        _BUILD_CACHE["nc"] = build()
    return _BUILD_CACHE["nc"]


def make_in_maps(x, y):
    in_maps = []
    for bi in range(B):
        a = np.ascontiguousarray(
            np.asarray(x)[bi].transpose(0, 2, 1).reshape(C, K)
        ).astype(np.float16)
        bm = np.asarray(y)[bi].reshape(C, M).astype(np.float16)
        in_maps.append({"a": a, "b": bm})
    return in_maps


def run(x, y, trace=False):
    from concourse import bass_utils

    nc = get_built()
    in_maps = make_in_maps(x, y)
    res = bass_utils.run_bass_kernel_spmd(
        nc, in_maps, core_ids=list(range(B)), trace=trace
    )
    out = np.empty((B, K, M), dtype=np.float32)
    for bi in range(B):
        out[bi] = res.results[bi]["out"].T  # [M,K] fp16 -> [K,M] fp32
    return out.reshape(B, K, H, W), res


def kernel(x, y):
    out, _ = run(x, y, trace=False)
    return out


# revision 12
# speedup vs baseline: 5.5365x; 5.5365x over previous
"""Correlation-layer kernel for Trainium2 (8 NeuronCores, data-parallel over batch).

Problem (per batch b):
    corr[k, m] = sum_c x[b, c, u, v] * y[b, c, i, j],  k = v*h+u, m = i*w+j
    out = relu(corr) / sqrt(sum_k relu(corr)^2 + eps)   (normalize over k per m)

Shapes: x, y = (8, 128, 48, 64) fp32 -> out (8, 3072, 48, 64) fp32.
Sharding: 1 batch per core.

Design (v4): transposed layout — m on partitions, k on the free dim.
Per m-tile (128 rows of m, 3072 k columns):
  - 6 fp16 matmuls (lhsT = y columns, rhs = x) -> two 3-bank psum tiles.
  - relu evacuation psum -> fp16 r16 tile, split ACT/DVE.
  - sum-of-squares over k via one fused DVE tensor_tensor_reduce
    (accum_out, eps folded as the reduce initial value).
  - s = 1/sqrt(ss): ACT sqrt + DVE reciprocal (+1 Newton step).
  - out16 = r16 * s (per-partition scalar): DVE 4x tensor_scalar +
    GpSimd slice.
  - one contiguous 786 KB HWDGE DMA per m-tile into a [M, K] fp16
    DRAM tensor.
Host side: transpose [M, K] -> [K, M], upcast fp16 -> fp32, stack batches.
Normalization over k is a per-partition (free-dim) op in this layout, which
kills the ones-matmul reduction, transpose chains, and broadcast matmul of
the previous design, and fp16 output halves HBM write traffic.
"""

import sys

sys.path.insert(0, "/opt/trn_rl_repo")

import numpy as np

_BUILD_CACHE = {}

B, C, H, W = 8, 128, 48, 64
K = W * H      # 3072 output channels, k = v*h+u
M = H * W      # 3072 spatial positions, m = i*w+j
NT = M // 128  # 24 m-tiles
EPS = 1e-6

PSA = 1536     # cols in first psum tile (ACT evacs all of it)
PSB = 1536     # cols in second psum tile
EV_B_ACT = 1024  # leading cols of psB evac'd by ACT (bank-aligned), rest DVE
SC_GPS = 0     # trailing cols of the scale pass on GpSimd (rest DVE).
               # GpSimd AP-scalar ops run ~27us AND poison DVE via SBUF
               # contention — keep 0.
NEWTON = False  # refine DVE reciprocal with one Newton step
SS_MODE = "stt"  # "stt": scalar_tensor_tensor w/ accum_out (1 DVE pass)
                 # "ttred": tensor_tensor + tensor_reduce (reduce is 1x, slow)
                 # "ttr": tensor_tensor_reduce — CRASHES on HW, sim only


def build():
    from concourse import bacc, bass, mybir, tile

    F32 = mybir.dt.float32
    F16 = mybir.dt.float16
    AF = mybir.ActivationFunctionType
    OP = mybir.AluOpType

    nc = bacc.Bacc("TRN2", debug=False, target_bir_lowering=False)

    a_d = nc.dram_tensor("a", [C, K], F16, kind="ExternalInput")
    b_d = nc.dram_tensor("b", [C, M], F16, kind="ExternalInput")
    out_d = nc.dram_tensor("out", [M, K], F16, kind="ExternalOutput")

    with tile.TileContext(nc) as tc:
        with (
            tc.tile_pool(name="pers", bufs=1) as pers,
            tc.tile_pool(name="rk", bufs=3) as rk,
            tc.tile_pool(name="sq", bufs=2) as sqp,
            tc.tile_pool(name="ok", bufs=3) as ok,
            tc.tile_pool(name="sm", bufs=4) as sm,
            tc.tile_pool(name="ps", bufs=1, space=bass.MemorySpace.PSUM) as ps,
        ):
            a_t = pers.tile([C, K], F16)
            b_t = pers.tile([C, M], F16)
            nc.sync.dma_start(b_t[:], b_d[:])
            nc.sync.dma_start(a_t[:], a_d[:])

            state = {}

            def emit_front(i):
                """matmuls + relu evac + fused square-reduce for m-tile i."""
                m0 = i * 128
                psA = ps.tile([128, PSA], F32, tag="psA")
                psB = ps.tile([128, PSB], F32, tag="psB")
                for j in range(PSA // 512):
                    nc.tensor.matmul(
                        psA[:, j * 512 : (j + 1) * 512],
                        b_t[:, m0 : m0 + 128],
                        a_t[:, j * 512 : (j + 1) * 512],
                        start=True, stop=True,
                    )
                for j in range(PSB // 512):
                    nc.tensor.matmul(
                        psB[:, j * 512 : (j + 1) * 512],
                        b_t[:, m0 : m0 + 128],
                        a_t[:, PSA + j * 512 : PSA + (j + 1) * 512],
                        start=True, stop=True,
                    )
                r16 = rk.tile([128, K], F16, tag="r16")
                nc.scalar.activation(r16[:, 0:PSA], psA[:], AF.Relu)
                if EV_B_ACT:
                    nc.scalar.activation(
                        r16[:, PSA : PSA + EV_B_ACT], psB[:, 0:EV_B_ACT], AF.Relu
                    )
                if EV_B_ACT < PSB:
                    nc.vector.tensor_scalar_max(
                        r16[:, PSA + EV_B_ACT : K], psB[:, EV_B_ACT : PSB], 0.0
                    )
                sq16 = sqp.tile([128, K], F16, tag="sq16")
                ss = sm.tile([128, 1], F32, tag="ss")
                if SS_MODE == "ttr":
                    nc.vector.tensor_tensor_reduce(
                        out=sq16[:], in0=r16[:], in1=r16[:], scale=1.0, scalar=EPS,
                        op0=OP.mult, op1=OP.add, accum_out=ss[:],
                    )
                elif SS_MODE == "stt":
                    # out = max(r16, 0) * r16 = r16^2 (r16 >= 0); accum = ss
                    ss_r = sm.tile([128, 1], F32, tag="ssr")
                    nc.vector.scalar_tensor_tensor(
                        out=sq16[:], in0=r16[:], scalar=0.0, in1=r16[:],
                        op0=OP.max, op1=OP.mult, accum_out=ss_r[:],
                    )
                    nc.vector.tensor_scalar_add(ss[:], ss_r[:], EPS)
                else:
                    nc.vector.tensor_tensor(sq16[:], r16[:], r16[:], OP.mult)
                    ss_r = sm.tile([128, 1], F32, tag="ssr")
                    nc.vector.tensor_reduce(
                        ss_r[:], sq16[:], mybir.AxisListType.X, OP.add
                    )
                    nc.vector.tensor_scalar_add(ss[:], ss_r[:], EPS)
                state[i] = (r16, ss)

            def emit_back(i):
                """rsqrt + scale + output DMA for m-tile i."""
                r16, ss = state.pop(i)
                s0 = sm.tile([128, 1], F32, tag="s0")
                nc.scalar.activation(s0[:], ss[:], AF.Sqrt)
                y0 = sm.tile([128, 1], F32, tag="y0")
                nc.vector.reciprocal(y0[:], s0[:])
                if NEWTON:
                    # y1 = y0 * (2 - s0*y0)
                    p = sm.tile([128, 1], F32, tag="p")
                    nc.vector.tensor_tensor(p[:], s0[:], y0[:], OP.mult)
                    q = sm.tile([128, 1], F32, tag="q")
                    nc.vector.tensor_scalar(
                        out=q[:], in0=p[:], scalar1=-1.0, scalar2=2.0,
                        op0=OP.mult, op1=OP.add,
                    )
                    s_fin = sm.tile([128, 1], F32, tag="sfin")
                    nc.vector.tensor_tensor(s_fin[:], y0[:], q[:], OP.mult)
                else:
                    s_fin = y0
                out16 = ok.tile([128, K], F16, tag="out16")
                nc.vector.tensor_scalar(
                    out=out16[:, 0 : K - SC_GPS], in0=r16[:, 0 : K - SC_GPS],
                    scalar1=s_fin[:], scalar2=None, op0=OP.mult,
                )
                if SC_GPS:
                    nc.gpsimd.tensor_scalar(
                        out=out16[:, K - SC_GPS : K], in0=r16[:, K - SC_GPS : K],
                        scalar1=s_fin[:], scalar2=None, op0=OP.mult,
                    )
                nc.sync.dma_start(out_d[i * 128 : (i + 1) * 128, :], out16[:])

            emit_front(0)
            for i in range(1, NT + 1):
                if i < NT:
                    emit_front(i)
                emit_back(i - 1)

    nc.compile()
    return nc


def get_built():
    if "nc" not in _BUILD_CACHE:
        _BUILD_CACHE["nc"] = build()
    return _BUILD_CACHE["nc"]


def make_in_maps(x, y):
    in_maps = []
    for bi in range(B):
        a = np.ascontiguousarray(
            np.asarray(x)[bi].transpose(0, 2, 1).reshape(C, K)
        ).astype(np.float16)
        bm = np.asarray(y)[bi].reshape(C, M).astype(np.float16)
        in_maps.append({"a": a, "b": bm})
    return in_maps


def run(x, y, trace=False):
    from concourse import bass_utils

    nc = get_built()
    in_maps = make_in_maps(x, y)
    res = bass_utils.run_bass_kernel_spmd(
        nc, in_maps, core_ids=list(range(B)), trace=trace
    )
    out = np.empty((B, K, M), dtype=np.float32)
    for bi in range(B):
        out[bi] = res.results[bi]["out"].T  # [M,K] fp16 -> [K,M] fp32
    return out.reshape(B, K, H, W), res


def kernel(x, y):
    out, _ = run(x, y, trace=False)
    return out


# revision 17
# speedup vs baseline: 6.4844x; 1.1712x over previous
"""Correlation-layer kernel for Trainium2 (8 NeuronCores, data-parallel over batch).

Problem (per batch b):
    corr[k, m] = sum_c x[b, c, u, v] * y[b, c, i, j],  k = v*h+u, m = i*w+j
    out = relu(corr) / sqrt(sum_k relu(corr)^2 + eps)   (normalize over k per m)

Shapes: x, y = (8, 128, 48, 64) fp32 -> out (8, 3072, 48, 64) fp32.
Sharding: 1 batch per core.

Layout: m on partitions, k on the free dim (normalization over k becomes a
free-dim reduce + per-partition scale; output DMA'd as [M, K] fp16 and
transposed/upcast on the host).

Design S ("fused squares"): per m-tile (128 m x 3072 k):
  - 6 fp16 matmuls -> two 3-bank psum tiles.
  - DVE pass (one per psum tile): sq16 = max(psum,0)*psum = relu^2 with
    accum_out = sum_k -> ss. This is the only psum read; psum recycles fast.
  - s2 = 1/(ss_a + ss_b + eps) via one small STT + DVE reciprocal.
  - ACT pass: out16 = Sqrt(sq16 * s2) = relu * rsqrt(ss) in ONE activation
    (per-partition scale operand).
  - one contiguous 786 KB HWDGE DMA per m-tile.
Each output element passes through DVE once and ACT once - the structural
minimum given that the norm couples all k.

Design A (fallback): ACT relu-evac + DVE STT square/accum from fp16 + DVE
4x scale pass.
"""

import sys

sys.path.insert(0, "/opt/trn_rl_repo")

import numpy as np

_BUILD_CACHE = {}

B, C, H, W = 8, 128, 48, 64
K = W * H      # 3072 output channels, k = v*h+u
M = H * W      # 3072 spatial positions, m = i*w+j
NT = M // 128  # 24 m-tiles
EPS = 1e-6

DESIGN = "A"
PSA = 1536     # cols in first psum tile
PSB = 1536     # cols in second psum tile
EV_B_ACT = 1024  # (A) leading cols of psB evac'd by ACT, rest DVE
SS_ACT = 832   # (A) trailing cols of the square+accum pass on ACT Square
NEWTON = False  # refine DVE reciprocal with one Newton step


def build():
    from concourse import bacc, bass, mybir, tile

    F32 = mybir.dt.float32
    F16 = mybir.dt.float16
    AF = mybir.ActivationFunctionType
    OP = mybir.AluOpType

    nc = bacc.Bacc("TRN2", debug=False, target_bir_lowering=False)

    a_d = nc.dram_tensor("a", [C, K], F16, kind="ExternalInput")
    b_d = nc.dram_tensor("b", [C, M], F16, kind="ExternalInput")
    out_d = nc.dram_tensor("out", [M, K], F16, kind="ExternalOutput")

    with tile.TileContext(nc) as tc:
        with (
            tc.tile_pool(name="pers", bufs=1) as pers,
            tc.tile_pool(name="rk", bufs=3) as rk,
            tc.tile_pool(name="sq", bufs=3) as sqp,
            tc.tile_pool(name="ok", bufs=3) as ok,
            tc.tile_pool(name="sm", bufs=4) as sm,
            tc.tile_pool(name="ps", bufs=1, space=bass.MemorySpace.PSUM) as ps,
        ):
            a_t = pers.tile([C, K], F16)
            b_t = pers.tile([C, M], F16)
            nc.sync.dma_start(b_t[:], b_d[:])
            nc.sync.dma_start(a_t[:], a_d[:])

            state = {}

            def matmuls(i):
                m0 = i * 128
                psA = ps.tile([128, PSA], F32, tag="psA")
                psB = ps.tile([128, PSB], F32, tag="psB")
                for j in range(PSA // 512):
                    nc.tensor.matmul(
                        psA[:, j * 512 : (j + 1) * 512],
                        b_t[:, m0 : m0 + 128],
                        a_t[:, j * 512 : (j + 1) * 512],
                        start=True, stop=True,
                    )
                for j in range(PSB // 512):
                    nc.tensor.matmul(
                        psB[:, j * 512 : (j + 1) * 512],
                        b_t[:, m0 : m0 + 128],
                        a_t[:, PSA + j * 512 : PSA + (j + 1) * 512],
                        start=True, stop=True,
                    )
                return psA, psB

            def emit_front_S(i):
                psA, psB = matmuls(i)
                sq16 = sqp.tile([128, K], F16, tag="sq16")
                ssa = sm.tile([128, 1], F32, tag="ssa")
                ssb = sm.tile([128, 1], F32, tag="ssb")
                nc.vector.scalar_tensor_tensor(
                    out=sq16[:, 0:PSA], in0=psA[:], scalar=0.0, in1=psA[:],
                    op0=OP.max, op1=OP.mult, accum_out=ssa[:],
                )
                nc.vector.scalar_tensor_tensor(
                    out=sq16[:, PSA:K], in0=psB[:], scalar=0.0, in1=psB[:],
                    op0=OP.max, op1=OP.mult, accum_out=ssb[:],
                )
                ss = sm.tile([128, 1], F32, tag="ss")
                nc.vector.scalar_tensor_tensor(
                    out=ss[:], in0=ssa[:], scalar=EPS, in1=ssb[:],
                    op0=OP.add, op1=OP.add,
                )
                s2 = sm.tile([128, 1], F32, tag="s2")
                nc.vector.reciprocal(s2[:], ss[:])
                if NEWTON:
                    p = sm.tile([128, 1], F32, tag="p")
                    nc.vector.tensor_tensor(p[:], ss[:], s2[:], OP.mult)
                    q = sm.tile([128, 1], F32, tag="q")
                    nc.vector.tensor_scalar(
                        out=q[:], in0=p[:], scalar1=-1.0, scalar2=2.0,
                        op0=OP.mult, op1=OP.add,
                    )
                    s2n = sm.tile([128, 1], F32, tag="s2n")
                    nc.vector.tensor_tensor(s2n[:], s2[:], q[:], OP.mult)
                    s2 = s2n
                state[i] = (sq16, s2)

            def emit_back_S(i):
                sq16, s2 = state.pop(i)
                out16 = ok.tile([128, K], F16, tag="out16")
                nc.scalar.activation(out16[:], sq16[:], AF.Sqrt, scale=s2[:])
                nc.sync.dma_start(out_d[i * 128 : (i + 1) * 128, :], out16[:])

            def emit_front_A(i):
                psA, psB = matmuls(i)
                r16 = rk.tile([128, K], F16, tag="r16")
                nc.scalar.activation(r16[:, 0:PSA], psA[:], AF.Relu)
                if EV_B_ACT:
                    nc.scalar.activation(
                        r16[:, PSA : PSA + EV_B_ACT], psB[:, 0:EV_B_ACT], AF.Relu
                    )
                if EV_B_ACT < PSB:
                    nc.vector.tensor_scalar_max(
                        r16[:, PSA + EV_B_ACT : K], psB[:, EV_B_ACT : PSB], 0.0
                    )
                sq16 = sqp.tile([128, K], F16, tag="sq16")
                ssd = sm.tile([128, 1], F32, tag="ssd")
                w0 = K - SS_ACT
                nc.vector.scalar_tensor_tensor(
                    out=sq16[:, 0:w0], in0=r16[:, 0:w0], scalar=0.0,
                    in1=r16[:, 0:w0], op0=OP.max, op1=OP.mult, accum_out=ssd[:],
                )
                ss = sm.tile([128, 1], F32, tag="ss")
                if SS_ACT:
                    ssa = sm.tile([128, 1], F32, tag="ssa")
                    nc.scalar.activation(
                        sq16[:, w0:K], r16[:, w0:K], AF.Square, accum_out=ssa[:]
                    )
                    nc.vector.scalar_tensor_tensor(
                        out=ss[:], in0=ssd[:], scalar=EPS, in1=ssa[:],
                        op0=OP.add, op1=OP.add,
                    )
                else:
                    nc.vector.tensor_scalar_add(ss[:], ssd[:], EPS)
                s0 = sm.tile([128, 1], F32, tag="s0")
                nc.scalar.activation(s0[:], ss[:], AF.Sqrt)
                s1 = sm.tile([128, 1], F32, tag="s1")
                nc.vector.reciprocal(s1[:], s0[:])
                state[i] = (r16, s1)

            def emit_back_A(i):
                r16, s1 = state.pop(i)
                out16 = ok.tile([128, K], F16, tag="out16")
                nc.vector.tensor_scalar(
                    out=out16[:], in0=r16[:], scalar1=s1[:], scalar2=None,
                    op0=OP.mult,
                )
                nc.sync.dma_start(out_d[i * 128 : (i + 1) * 128, :], out16[:])

            front = emit_front_S if DESIGN == "S" else emit_front_A
            back = emit_back_S if DESIGN == "S" else emit_back_A

            # back(i-1) first: its DVE scale op fills the gap while DVE
            # waits on ACT's relu evacs for tile i.
            for i in range(NT + 1):
                if i >= 1:
                    back(i - 1)
                if i < NT:
                    front(i)

    nc.compile()
    return nc


def get_built():
    if "nc" not in _BUILD_CACHE:
        _BUILD_CACHE["nc"] = build()
    return _BUILD_CACHE["nc"]


def make_in_maps(x, y):
    in_maps = []
    for bi in range(B):
        a = np.ascontiguousarray(
            np.asarray(x)[bi].transpose(0, 2, 1).reshape(C, K)
        ).astype(np.float16)
        bm = np.asarray(y)[bi].reshape(C, M).astype(np.float16)
        in_maps.append({"a": a, "b": bm})
    return in_maps


def run(x, y, trace=False):
    from concourse import bass_utils

    nc = get_built()
    in_maps = make_in_maps(x, y)
    res = bass_utils.run_bass_kernel_spmd(
        nc, in_maps, core_ids=list(range(B)), trace=trace
    )
    out = np.empty((B, K, M), dtype=np.float32)
    for bi in range(B):
        out[bi] = res.results[bi]["out"].T  # [M,K] fp16 -> [K,M] fp32
    return out.reshape(B, K, H, W), res


def kernel(x, y):
    out, _ = run(x, y, trace=False)
    return out
